# revision 10
# baseline (speedup 1.0000x reference)
"""DeformConv2d (B=8, C=128, H=W=64, K=3x3, pad 1, stride 1) on 8 trn2 NeuronCores.

Data-parallel over batch: core b handles image b. Per core:
  - Host packs x into a position-major DRAM image xpm[NE, 512] bf16: row i
    holds the 2x2 bilinear patch at padded pixel i as 4 channel-blocks
    [A=x(i), C=x(i+68), B=x(i+1), D=x(i+69)] x 128 channels.
  - Host precomputes the wrapped-16 gather indices (idxw) and the per-tap
    bilinear corner weights (wrow) directly in device layout, so the kernel
    has no index/weight prologue at all.
  - Per (quarter, tap): SWDGE dma_gather (transpose mode, 4 queues) fetches
    1024 rows of 1KB into [128ch, 4 corner-blocks, 1024 pos] bf16; bilinear
    corner weights are broadcast stride-0 on the HWDGE rings; DVE multiply;
    PE matmuls accumulate 9 taps into two position-half PSUM tiles.
  - Tail per (quarter, half): sum 4 corner lanes + bias -> fp32 out.
"""
import numpy as np
import ml_dtypes

B, CIN, H, W = 8, 128, 64, 64
COUT, KH, KW = 128, 3, 3
K = KH * KW
HO, WO = 64, 64
P = 128                      # partitions
NPOS = HO * WO               # 4096 output positions per image
Q = NPOS // P                # 32 position-blocks of 128
PADR = 2                     # zero-pad ring width
HP = H + 2 * PADR            # 68
WP = W + 2 * PADR            # 68
NE = HP * WP                 # 4624 padded pixels
ES = 4 * P                   # 512 bf16 elements per xpm row (1KB)
NQT = NPOS // 4              # 1024 positions per quarter
NH = NQT // 2                # 512 positions per half


def _build_kernel():
    import concourse.bacc as bacc
    import concourse.mybir as mybir
    import concourse.tile as tile
    import concourse.library_config as library_config

    nc = bacc.Bacc("TRN2", target_bir_lowering=False, debug=False,
                   num_devices=8, num_swdge_queues=4)
    f32, bf16, i16 = mybir.dt.float32, mybir.dt.bfloat16, mybir.dt.int16
    ALU = mybir.AluOpType

    xpm_d = nc.dram_tensor("xpm", [NE, ES], bf16, kind="ExternalInput")
    idxw_d = nc.dram_tensor("idxw", [P, 8 * K * Q], i16, kind="ExternalInput")
    wrow_d = nc.dram_tensor("wrow", [K, 4 * NPOS], bf16, kind="ExternalInput")
    wmat_d = nc.dram_tensor("wmat", [P, K * COUT], bf16, kind="ExternalInput")
    bias_d = nc.dram_tensor("bias", [P, 1], f32, kind="ExternalInput")
    out_d = nc.dram_tensor("out", [P, NPOS], f32, kind="ExternalOutput")

    NI = 8 * K * Q  # 2304 idx-cols (256 per tap: 64 per (qt,k))

    with tile.TileContext(nc) as tc:
        with tc.tile_pool(name="const", bufs=1) as cpool, \
             tc.tile_pool(name="wbc", bufs=6) as wpool, \
             tc.tile_pool(name="gath", bufs=6) as gapool, \
             tc.tile_pool(name="mm", bufs=3) as mpool, \
             tc.tile_pool(name="outp", bufs=2) as opool, \
             tc.tile_pool(name="ps", bufs=2, space="PSUM") as pspool:

            nc.gpsimd.load_library(library_config.mlp)
            gsems = [nc.alloc_semaphore(f"gsem{qn}") for qn in range(4)]

            # ---------------- input loads ------------------------------
            idxw = cpool.tile([P, NI], i16)
            nc.sync.dma_start(out=idxw[:], in_=idxw_d.ap())
            wmat = cpool.tile([P, K * COUT], bf16)
            nc.scalar.dma_start(out=wmat[:], in_=wmat_d.ap())
            bias = cpool.tile([P, 1], f32)
            nc.sync.dma_start(out=bias[:], in_=bias_d.ap())

            # ---------------- main loop: quarters x taps ----------------
            rr = 0
            for qt in range(4):
                psA = pspool.tile([P, 4 * NH], mybir.dt.float32, tag="ps")
                psB = pspool.tile([P, 4 * NH], mybir.dt.float32, tag="ps")
                ps = [psA, psB]
                for k in range(K):
                    sl = slice(qt * 4 * NQT, (qt + 1) * 4 * NQT)
                    with tc.high_priority():
                        wb = wpool.tile([P, 4 * NQT], bf16, tag="wb")
                        eng = nc.sync if rr % 2 == 0 else nc.scalar
                        eng.dma_start(
                            out=wb[:],
                            in_=wrow_d.ap()[k: k + 1, sl].to_broadcast(
                                (P, 4 * NQT)))
                    g4 = gapool.tile([P, 4 * NQT], bf16, tag="g")
                    i0 = k * 256 + qt * 64
                    qn, mq = rr % 4, rr // 4
                    nc.gpsimd.dma_gather(
                        g4[:].rearrange("p (b n) -> p b n", b=4),
                        xpm_d.ap(), idxw[:, i0: i0 + 64],
                        num_idxs=NQT, num_idxs_reg=NQT,
                        elem_size=ES, transpose=True,
                        queue_num=qn, single_packet=False,
                        prepare_only=True, sem=gsems[qn])
                    trig = nc.gpsimd.trigger_dma(count=None, queue_num=qn)
                    if mq > 0:
                        trig._wait_ge(gsems[qn], 16 * mq)
                    m = mpool.tile([P, 4 * NQT], bf16, tag="m")
                    mul = nc.vector.tensor_tensor(
                        out=m[:], in0=g4[:], in1=wb[:], op=ALU.mult)
                    # prepare_only defers the gather write: gate the consumer
                    # on the DMA completion sem (16 per finished gather).
                    mul._wait_ge(gsems[qn], 16 * (mq + 1))
                    rr += 1
                    lhsT = wmat[:, k * COUT: (k + 1) * COUT]
                    for j in range(4):
                        for h in range(2):
                            nc.tensor.matmul(
                                ps[h][:, j * NH: (j + 1) * NH], lhsT,
                                m[:, j * NQT + h * NH: j * NQT + (h + 1) * NH],
                                start=(k == 0), stop=(k == K - 1),
                                skip_group_check=True)
                # tails: sum 4 corner lanes + bias -> fp32 out (per half)
                for h in range(2):
                    pv = ps[h][:].rearrange("o (j n) -> o j n", j=4)
                    with tc.high_priority():
                        t = opool.tile([P, NH], f32, tag="t")
                        nc.vector.tensor_scalar(
                            out=t[:], in0=pv[:, 0, :],
                            scalar1=bias[:, 0:1], scalar2=None, op0=ALU.add)
                        t2 = opool.tile([P, NH], f32, tag="t2")
                        nc.vector.tensor_tensor(
                            out=t2[:], in0=t[:], in1=pv[:, 1, :], op=ALU.add)
                        t3 = opool.tile([P, NH], f32, tag="t3")
                        nc.vector.tensor_tensor(
                            out=t3[:], in0=t2[:], in1=pv[:, 2, :], op=ALU.add)
                        # final op reorders i' = 8p+q  ->  (q, p) blocks
                        o = opool.tile([P, NH], f32, tag="o")
                        nc.vector.tensor_tensor(
                            out=o[:].rearrange("o (q p) -> o p q", q=8, p=64),
                            in0=t3[:].rearrange("o (p q) -> o p q", p=64, q=8),
                            in1=pv[:, 3, :].rearrange("o (p q) -> o p q",
                                                      p=64, q=8),
                            op=ALU.add)
                    # store: o[(q,p)] -> out[qt*1024 + q*128 + h*64 + p]
                    eng = nc.sync if h == 0 else nc.scalar
                    eng.dma_start(
                        out=out_d.ap().rearrange(
                            "o (qt q hh p) -> o qt q hh p",
                            qt=4, q=8, hh=2, p=64)[:, qt, :, h, :],
                        in_=o[:].rearrange("o (q p) -> o q p", q=8, p=64))

    nc.compile()
    return nc


_NC_CACHE = None


def _host_inputs(x, offset, weight, bias):
    """Per-core input maps (core b <- batch b) + replicated constants."""
    wq = np.ascontiguousarray(weight, np.float32)  # [COUT, CIN, KH, KW]
    wmat = wq.reshape(COUT, CIN, K).transpose(1, 2, 0).reshape(CIN, K * COUT)
    wmat = np.ascontiguousarray(wmat).astype(ml_dtypes.bfloat16)
    bias_h = np.ascontiguousarray(bias, np.float32).reshape(P, 1)

    # base sampling positions per (k, pos), f32 to match device/reference
    ho = (np.arange(NPOS, dtype=np.int32) // WO).astype(np.float32)  # [NPOS]
    wo = (np.arange(NPOS, dtype=np.int32) % WO).astype(np.float32)
    ky = (np.arange(K, dtype=np.int32) // 3 - 1).astype(np.float32)  # [K]
    kx = (np.arange(K, dtype=np.int32) % 3 - 1).astype(np.float32)

    in_maps = []
    for b in range(B):
        img = np.ascontiguousarray(x[b], np.float32).transpose(1, 2, 0)
        XPf = np.zeros((HP, WP, P), np.float32)
        XPf[PADR:PADR + H, PADR:PADR + W] = img
        ext = np.vstack([XPf.reshape(NE, P), np.zeros((WP + 1, P), np.float32)])
        xpm = np.concatenate(
            [ext[0:NE], ext[WP:NE + WP], ext[1:NE + 1], ext[WP + 1:NE + WP + 1]],
            axis=1).astype(ml_dtypes.bfloat16)  # [NE, 512]

        offb = np.ascontiguousarray(offset[b], np.float32).reshape(2 * K, NPOS)
        py = (ky[:, None] + ho[None, :]) + offb[0::2]   # [K, NPOS] f32
        px = (kx[:, None] + wo[None, :]) + offb[1::2]
        y0 = np.floor(py)
        x0 = np.floor(px)
        ly = py - y0
        lx = px - x0
        y0c = np.clip(y0, -PADR, 64.0)
        x0c = np.clip(x0, -PADR, 64.0)
        lin = ((y0c + PADR) * WP + (x0c + PADR)).astype(np.int16)  # [K, NPOS]

        # idxw[16*g + 8*pm + q, k*256 + qt*64 + ph]
        #   = lin[k, (qt*8+q)*128 + 2*ph + pm]
        lin_r = lin.reshape(K, 4, 8, 64, 2)           # k qt q ph pm
        idx16 = lin_r.transpose(4, 2, 0, 1, 3).reshape(16, K * 4 * 64)
        idxw = np.ascontiguousarray(np.tile(idx16, (8, 1)))  # [128, 2304]

        # wrow[k, qt*4096 + j*1024 + p*8 + q] = w_j[k, (qt*8+q)*128 + p]
        w4 = np.stack([(1 - ly) * (1 - lx), ly * (1 - lx),
                       (1 - ly) * lx, ly * lx])        # j k pos
        w4r = w4.reshape(4, K, 4, 8, P)                # j k qt q p
        wrow = np.ascontiguousarray(
            w4r.transpose(1, 2, 0, 4, 3).reshape(K, 4 * NPOS)
        ).astype(ml_dtypes.bfloat16)

        in_maps.append({
            "xpm": xpm,
            "idxw": idxw,
            "wrow": wrow,
            "wmat": wmat,
            "bias": bias_h,
        })
    return in_maps


def kernel(x, offset, weight, bias):
    global _NC_CACHE
    from concourse.bass_utils import run_bass_kernel_spmd

    if _NC_CACHE is None:
        _NC_CACHE = _build_kernel()
    nc = _NC_CACHE
    in_maps = _host_inputs(x, offset, weight, bias)
    res = run_bass_kernel_spmd(nc, in_maps, list(range(B)))
    out = np.stack([res.results[b]["out"].reshape(COUT, HO, WO) for b in range(B)])
    return out.astype(np.float32)


if __name__ == "__main__":
    import sys
    d = np.load("/tmp/inputs.npz")
    if len(sys.argv) > 1 and sys.argv[1] == "sim":
        from concourse.bass_interp import CoreSim
        nc = _build_kernel()
        in_maps = _host_inputs(d["x"], d["offset"], d["weight"], d["bias"])
        sim = CoreSim(nc)
        for kk, vv in in_maps[0].items():
            sim.tensor(kk)[:] = vv
        sim.simulate()
        out = np.asarray(sim.tensor("out")).reshape(1, COUT, HO, WO)
        exp = np.load("/tmp/expected.npy")[0:1]
    else:
        out = kernel(d["x"], d["offset"], d["weight"], d["bias"])
        exp = np.load("/tmp/expected.npy")
    err = np.abs(out - exp)
    print("rel l2:", np.linalg.norm(out - exp) / np.linalg.norm(exp))
    print("absmax rel:", err.max() / np.abs(exp).max())


# revision 11
# speedup vs baseline: 1.0311x; 1.0311x over previous
"""DeformConv2d (B=8, C=128, H=W=64, K=3x3, pad 1, stride 1) on 8 trn2 NeuronCores.

Data-parallel over batch: core b handles image b. Transposed-gather design:
  - Host packs x into xpm[NE, 512] bf16: row i = 2x2 bilinear patch at padded
    pixel i as 4 channel-blocks [A, C, B, D] x 128 ch.
  - Host precomputes wrapped-16 indices (idxc) ordered so gather ordinal
    i = pos-within-quarter, and compact per-position corner weights
    wtr[p, (k,qt,b,j)] -- no weight broadcast DMA at all.
  - Per (quarter, tap): SWDGE dma_gather transpose=False (prepare_only,
    round-robin over 4 queues, explicit trigger) fetches 1024 rows of 1KB
    into [128 pos, 8 blk, (4 corner x 128 ch)] bf16. DVE multiplies by
    corner weights (free-dim stride-0 broadcast) and sums the 4 corners.
    PE transposes each 128-pos block to channel-major (PSUM), ACT copies
    back to SBUF, PE matmul accumulates 9 taps into psum[cout, 1024].
  - Tail per quarter: bias add -> fp32 out (contiguous store).
"""
import numpy as np
import ml_dtypes

B, CIN, H, W = 8, 128, 64, 64
COUT, KH, KW = 128, 3, 3
K = KH * KW
HO, WO = 64, 64
P = 128
NPOS = HO * WO               # 4096
PADR = 2
HP = H + 2 * PADR            # 68
WP = W + 2 * PADR            # 68
NE = HP * WP                 # 4624
ES = 4 * P                   # 512 bf16 per xpm row (1KB)
NQT = NPOS // 4              # 1024 positions per quarter
NB = NQT // P                # 8 position-blocks per quarter


def _build_kernel():
    import concourse.bacc as bacc
    import concourse.mybir as mybir
    import concourse.tile as tile
    import concourse.library_config as library_config

    nc = bacc.Bacc("TRN2", target_bir_lowering=False, debug=False,
                   num_devices=8, num_swdge_queues=4)
    f32, bf16, i16 = mybir.dt.float32, mybir.dt.bfloat16, mybir.dt.int16
    ALU = mybir.AluOpType

    xpm_d = nc.dram_tensor("xpm", [NE, ES], bf16, kind="ExternalInput")
    idxc_d = nc.dram_tensor("idxc", [P, K * 4 * 64], i16, kind="ExternalInput")
    wtr_d = nc.dram_tensor("wtr", [P, K * 4 * NB * 4], bf16, kind="ExternalInput")
    wmat_d = nc.dram_tensor("wmat", [P, K * COUT], bf16, kind="ExternalInput")
    bias_d = nc.dram_tensor("bias", [P, 1], f32, kind="ExternalInput")
    ident_d = nc.dram_tensor("ident", [P, P], bf16, kind="ExternalInput")
    out_d = nc.dram_tensor("out", [P, NPOS], f32, kind="ExternalOutput")

    NI = K * 4 * 64  # 2304

    with tile.TileContext(nc) as tc:
        with tc.tile_pool(name="const", bufs=1) as cpool, \
             tc.tile_pool(name="gath", bufs=6) as gapool, \
             tc.tile_pool(name="m1p", bufs=3) as m1pool, \
             tc.tile_pool(name="rp", bufs=3) as rpool, \
             tc.tile_pool(name="mtp", bufs=3) as mtpool, \
             tc.tile_pool(name="outp", bufs=2) as opool, \
             tc.tile_pool(name="ps", bufs=2, space="PSUM") as pspool:

            nc.gpsimd.load_library(library_config.mlp)
            gsems = [nc.alloc_semaphore(f"gsem{qn}") for qn in range(4)]

            idxc = cpool.tile([P, NI], i16)
            nc.sync.dma_start(out=idxc[:], in_=idxc_d.ap())
            wtr = cpool.tile([P, K * 4 * NB * 4], bf16)
            nc.scalar.dma_start(out=wtr[:], in_=wtr_d.ap())
            wmat = cpool.tile([P, K * COUT], bf16)
            nc.scalar.dma_start(out=wmat[:], in_=wmat_d.ap())
            bias = cpool.tile([P, 1], f32)
            nc.sync.dma_start(out=bias[:], in_=bias_d.ap())
            ident = cpool.tile([P, P], bf16)
            nc.sync.dma_start(out=ident[:], in_=ident_d.ap())

            rr = 0
            for qt in range(4):
                acc = pspool.tile([P, NQT], mybir.dt.float32, tag="acc")
                for k in range(K):
                    qn, mq = rr % 4, rr // 4
                    g = gapool.tile([P, NB * ES], bf16, tag="g")
                    i0 = k * 256 + qt * 64
                    nc.gpsimd.dma_gather(
                        g[:].rearrange("p (b e) -> p b e", b=NB),
                        xpm_d.ap(), idxc[:, i0: i0 + 64],
                        num_idxs=NQT, num_idxs_reg=NQT,
                        elem_size=ES, transpose=False,
                        queue_num=qn, single_packet=False,
                        prepare_only=True, sem=gsems[qn])
                    trig = nc.gpsimd.trigger_dma(count=None, queue_num=qn)
                    if mq > 0:
                        trig._wait_ge(gsems[qn], 16 * mq)

                    # weights: wtr[:, (k,qt,b,j)] broadcast over ch (stride 0)
                    wsl = wtr[:, (k * 4 + qt) * NB * 4:
                              (k * 4 + qt + 1) * NB * 4]
                    m1 = m1pool.tile([P, NB * ES], bf16, tag="m1")
                    mul = nc.vector.tensor_tensor(
                        out=m1[:].rearrange("p (b j e) -> p b j e", b=NB, j=4),
                        in0=g[:].rearrange("p (b j e) -> p b j e", b=NB, j=4),
                        in1=wsl.rearrange("p (b j) -> p b j", b=NB)
                            .to_broadcast((P, NB, 4, P)),
                        op=ALU.mult)
                    mul._wait_ge(gsems[qn], 16 * (mq + 1))

                    m1v = m1[:].rearrange("p (b j e) -> p b j e", b=NB, j=4)
                    r1 = rpool.tile([P, NQT], bf16, tag="r1")
                    nc.vector.tensor_tensor(
                        out=r1[:].rearrange("p (b e) -> p b e", b=NB),
                        in0=m1v[:, :, 0], in1=m1v[:, :, 1], op=ALU.add)
                    r2 = rpool.tile([P, NQT], bf16, tag="r2")
                    nc.vector.tensor_tensor(
                        out=r2[:].rearrange("p (b e) -> p b e", b=NB),
                        in0=m1v[:, :, 2], in1=m1v[:, :, 3], op=ALU.add)
                    m2 = rpool.tile([P, NQT], bf16, tag="m2")
                    nc.vector.tensor_tensor(
                        out=m2[:], in0=r1[:], in1=r2[:], op=ALU.add)

                    # PE transpose per 128-pos block -> [ch, pos] in PSUM
                    pst = pspool.tile([P, NQT], bf16, tag="tr")
                    for b in range(NB):
                        nc.tensor.transpose(
                            pst[:, b * P: (b + 1) * P],
                            m2[:, b * P: (b + 1) * P], ident[:])
                    mt = mtpool.tile([P, NQT], bf16, tag="mt")
                    nc.scalar.copy(out=mt[:], in_=pst[:])

                    lhsT = wmat[:, k * COUT: (k + 1) * COUT]
                    for h in range(2):
                        nc.tensor.matmul(
                            acc[:, h * 512: (h + 1) * 512], lhsT,
                            mt[:, h * 512: (h + 1) * 512],
                            start=(k == 0), stop=(k == K - 1),
                            skip_group_check=True)
                    rr += 1

                ov = opool.tile([P, NQT], f32, tag="o")
                nc.vector.tensor_scalar(
                    out=ov[:], in0=acc[:], scalar1=bias[:, 0:1],
                    scalar2=None, op0=ALU.add)
                eng = nc.sync if qt % 2 == 0 else nc.scalar
                eng.dma_start(
                    out=out_d.ap()[:, qt * NQT: (qt + 1) * NQT], in_=ov[:])

    nc.compile()
    return nc


_NC_CACHE = None


def _host_inputs(x, offset, weight, bias):
    """Per-core input maps (core b <- batch b) + replicated constants."""
    wq = np.ascontiguousarray(weight, np.float32)  # [COUT, CIN, KH, KW]
    wmat = wq.reshape(COUT, CIN, K).transpose(1, 2, 0).reshape(CIN, K * COUT)
    wmat = np.ascontiguousarray(wmat).astype(ml_dtypes.bfloat16)
    bias_h = np.ascontiguousarray(bias, np.float32).reshape(P, 1)
    ident = np.eye(P, dtype=ml_dtypes.bfloat16)

    ho = (np.arange(NPOS, dtype=np.int32) // WO).astype(np.float32)
    wo = (np.arange(NPOS, dtype=np.int32) % WO).astype(np.float32)
    ky = (np.arange(K, dtype=np.int32) // 3 - 1).astype(np.float32)
    kx = (np.arange(K, dtype=np.int32) % 3 - 1).astype(np.float32)

    in_maps = []
    for b in range(B):
        img = np.ascontiguousarray(x[b], np.float32).transpose(1, 2, 0)
        XPf = np.zeros((HP, WP, P), np.float32)
        XPf[PADR:PADR + H, PADR:PADR + W] = img
        ext = np.vstack([XPf.reshape(NE, P), np.zeros((WP + 1, P), np.float32)])
        xpm = np.concatenate(
            [ext[0:NE], ext[WP:NE + WP], ext[1:NE + 1], ext[WP + 1:NE + WP + 1]],
            axis=1).astype(ml_dtypes.bfloat16)  # [NE, 512]

        offb = np.ascontiguousarray(offset[b], np.float32).reshape(2 * K, NPOS)
        py = (ky[:, None] + ho[None, :]) + offb[0::2]   # [K, NPOS] f32
        px = (kx[:, None] + wo[None, :]) + offb[1::2]
        y0 = np.floor(py)
        x0 = np.floor(px)
        ly = py - y0
        lx = px - x0
        y0c = np.clip(y0, -PADR, 64.0)
        x0c = np.clip(x0, -PADR, 64.0)
        lin = ((y0c + PADR) * WP + (x0c + PADR)).astype(np.int16)  # [K, NPOS]

        # idxc[s + 16g, k*256 + qt*64 + t] = lin[k, qt*1024 + 16t + s]
        lin_r = lin.reshape(K, 4, 64, 16)             # k qt t s
        idx16 = lin_r.transpose(3, 0, 1, 2).reshape(16, K * 4 * 64)
        idxc = np.ascontiguousarray(np.tile(idx16, (8, 1)))  # [128, 2304]

        # wtr[p, ((k*4+qt)*NB + b)*4 + j] = w_j[k, qt*1024 + b*128 + p]
        w4 = np.stack([(1 - ly) * (1 - lx), ly * (1 - lx),
                       (1 - ly) * lx, ly * lx])        # j k pos
        w4r = w4.reshape(4, K, 4, NB, P)               # j k qt b p
        wtr = np.ascontiguousarray(
            w4r.transpose(4, 1, 2, 3, 0).reshape(P, K * 4 * NB * 4)
        ).astype(ml_dtypes.bfloat16)

        in_maps.append({
            "xpm": xpm,
            "idxc": idxc,
            "wtr": wtr,
            "wmat": wmat,
            "bias": bias_h,
            "ident": ident,
        })
    return in_maps


def kernel(x, offset, weight, bias):
    global _NC_CACHE
    from concourse.bass_utils import run_bass_kernel_spmd

    if _NC_CACHE is None:
        _NC_CACHE = _build_kernel()
    nc = _NC_CACHE
    in_maps = _host_inputs(x, offset, weight, bias)
    res = run_bass_kernel_spmd(nc, in_maps, list(range(B)))
    out = np.stack([res.results[b]["out"].reshape(COUT, HO, WO) for b in range(B)])
    return out.astype(np.float32)


if __name__ == "__main__":
    import sys
    d = np.load("/tmp/inputs.npz")
    if len(sys.argv) > 1 and sys.argv[1] == "sim":
        from concourse.bass_interp import CoreSim
        nc = _build_kernel()
        in_maps = _host_inputs(d["x"], d["offset"], d["weight"], d["bias"])
        sim = CoreSim(nc)
        for kk, vv in in_maps[0].items():
            sim.tensor(kk)[:] = vv
        sim.simulate()
        out = np.asarray(sim.tensor("out")).reshape(1, COUT, HO, WO)
        exp = np.load("/tmp/expected.npy")[0:1]
    else:
        out = kernel(d["x"], d["offset"], d["weight"], d["bias"])
        exp = np.load("/tmp/expected.npy")
    err = np.abs(out - exp)
    print("rel l2:", np.linalg.norm(out - exp) / np.linalg.norm(exp))
    print("absmax rel:", err.max() / np.abs(exp).max())


# revision 12
# speedup vs baseline: 1.3119x; 1.2722x over previous
"""DeformConv2d (B=8, C=128, H=W=64, K=3x3, pad 1, stride 1) on 8 trn2 NeuronCores.

Data-parallel over batch: core b handles image b. Transposed-gather design:
  - Host packs x into xpm[NE, 512] bf16: row i = 2x2 bilinear patch at padded
    pixel i as 4 channel-blocks [A, C, B, D] x 128 ch.
  - Host precomputes wrapped-16 indices (idxc) ordered so gather ordinal
    i = pos-within-quarter, and compact per-position corner weights
    wtr[p, (k,qt,b,j)] -- no weight broadcast DMA at all.
  - Per (quarter, tap): SWDGE dma_gather transpose=False (prepare_only,
    round-robin over 4 queues, explicit trigger) fetches 1024 rows of 1KB
    into [128 pos, 8 blk, (4 corner x 128 ch)] bf16. DVE multiplies by
    corner weights (free-dim stride-0 broadcast) and sums the 4 corners.
    PE transposes each 128-pos block to channel-major (PSUM), ACT copies
    back to SBUF, PE matmul accumulates 9 taps into psum[cout, 1024].
  - Tail per quarter: bias add -> fp32 out (contiguous store).
"""
import numpy as np
import ml_dtypes

B, CIN, H, W = 8, 128, 64, 64
COUT, KH, KW = 128, 3, 3
K = KH * KW
HO, WO = 64, 64
P = 128
NPOS = HO * WO               # 4096
PADR = 2
HP = H + 2 * PADR            # 68
WP = W + 2 * PADR            # 68
NE = HP * WP                 # 4624
ES = 4 * P                   # 512 bf16 per xpm row (1KB)
NQT = NPOS // 4              # 1024 positions per quarter
NB = NQT // P                # 8 position-blocks per quarter


def _build_kernel():
    import concourse.bacc as bacc
    import concourse.mybir as mybir
    import concourse.tile as tile
    import concourse.library_config as library_config

    nc = bacc.Bacc("TRN2", target_bir_lowering=False, debug=False,
                   num_devices=8, num_swdge_queues=4)
    f32, bf16, i16 = mybir.dt.float32, mybir.dt.bfloat16, mybir.dt.int16
    ALU = mybir.AluOpType

    xpm_d = nc.dram_tensor("xpm", [NE, ES], bf16, kind="ExternalInput")
    idxc_d = nc.dram_tensor("idxc", [P, K * 4 * 64], i16, kind="ExternalInput")
    wtr_d = nc.dram_tensor("wtr", [P, K * 4 * NB * 4], bf16, kind="ExternalInput")
    wmat_d = nc.dram_tensor("wmat", [P, K * COUT], bf16, kind="ExternalInput")
    bias_d = nc.dram_tensor("bias", [P, 1], f32, kind="ExternalInput")
    ident_d = nc.dram_tensor("ident", [P, P], bf16, kind="ExternalInput")
    out_d = nc.dram_tensor("out", [P, NPOS], f32, kind="ExternalOutput")

    NI = K * 4 * 64  # 2304

    with tile.TileContext(nc) as tc:
        with tc.tile_pool(name="const", bufs=1) as cpool, \
             tc.tile_pool(name="gath", bufs=6) as gapool, \
             tc.tile_pool(name="m1p", bufs=3) as m1pool, \
             tc.tile_pool(name="rp", bufs=3) as rpool, \
             tc.tile_pool(name="mtp", bufs=3) as mtpool, \
             tc.tile_pool(name="outp", bufs=2) as opool, \
             tc.tile_pool(name="ps", bufs=2, space="PSUM") as pspool:

            nc.gpsimd.load_library(library_config.mlp)

            idxc = cpool.tile([P, NI], i16)
            nc.sync.dma_start(out=idxc[:], in_=idxc_d.ap())
            wtr = cpool.tile([P, K * 4 * NB * 4], bf16)
            nc.scalar.dma_start(out=wtr[:], in_=wtr_d.ap())
            wmat = cpool.tile([P, K * COUT], bf16)
            nc.scalar.dma_start(out=wmat[:], in_=wmat_d.ap())
            bias = cpool.tile([P, 1], f32)
            nc.sync.dma_start(out=bias[:], in_=bias_d.ap())
            ident = cpool.tile([P, P], bf16)
            nc.sync.dma_start(out=ident[:], in_=ident_d.ap())

            rr = 0
            for qt in range(4):
                acc = pspool.tile([P, NQT], mybir.dt.float32, tag="acc")
                for k in range(K):
                    qn = rr % 4
                    g = gapool.tile([P, NB * ES], bf16, tag="g")
                    i0 = k * 256 + qt * 64
                    nc.gpsimd.dma_gather(
                        g[:].rearrange("p (b e) -> p b e", b=NB),
                        xpm_d.ap(), idxc[:, i0: i0 + 64],
                        num_idxs=NQT, num_idxs_reg=NQT,
                        elem_size=ES, transpose=False,
                        queue_num=qn, single_packet=False)

                    # weights: wtr[:, (k,qt,b,j)] broadcast over ch (stride 0)
                    wsl = wtr[:, (k * 4 + qt) * NB * 4:
                              (k * 4 + qt + 1) * NB * 4]
                    m1 = m1pool.tile([P, NB * ES], bf16, tag="m1")
                    nc.vector.tensor_tensor(
                        out=m1[:].rearrange("p (b j e) -> p b j e", b=NB, j=4),
                        in0=g[:].rearrange("p (b j e) -> p b j e", b=NB, j=4),
                        in1=wsl.rearrange("p (b j) -> p b j", b=NB)
                            .to_broadcast((P, NB, 4, P)),
                        op=ALU.mult)

                    m1v = m1[:].rearrange("p (b j e) -> p b j e", b=NB, j=4)
                    r1 = rpool.tile([P, NQT], bf16, tag="r1")
                    nc.vector.tensor_tensor(
                        out=r1[:].rearrange("p (b e) -> p b e", b=NB),
                        in0=m1v[:, :, 0], in1=m1v[:, :, 1], op=ALU.add)
                    r2 = rpool.tile([P, NQT], bf16, tag="r2")
                    nc.vector.tensor_tensor(
                        out=r2[:].rearrange("p (b e) -> p b e", b=NB),
                        in0=m1v[:, :, 2], in1=m1v[:, :, 3], op=ALU.add)
                    m2 = rpool.tile([P, NQT], bf16, tag="m2")
                    nc.vector.tensor_tensor(
                        out=m2[:], in0=r1[:], in1=r2[:], op=ALU.add)

                    # PE transpose per 128-pos block -> [ch, pos] in PSUM
                    pst = pspool.tile([P, NQT], bf16, tag="tr")
                    for b in range(NB):
                        nc.tensor.transpose(
                            pst[:, b * P: (b + 1) * P],
                            m2[:, b * P: (b + 1) * P], ident[:])
                    mt = mtpool.tile([P, NQT], bf16, tag="mt")
                    nc.scalar.copy(out=mt[:], in_=pst[:])

                    lhsT = wmat[:, k * COUT: (k + 1) * COUT]
                    for h in range(2):
                        nc.tensor.matmul(
                            acc[:, h * 512: (h + 1) * 512], lhsT,
                            mt[:, h * 512: (h + 1) * 512],
                            start=(k == 0), stop=(k == K - 1),
                            skip_group_check=True)
                    rr += 1

                ov = opool.tile([P, NQT], f32, tag="o")
                nc.vector.tensor_scalar(
                    out=ov[:], in0=acc[:], scalar1=bias[:, 0:1],
                    scalar2=None, op0=ALU.add)
                eng = nc.sync if qt % 2 == 0 else nc.scalar
                eng.dma_start(
                    out=out_d.ap()[:, qt * NQT: (qt + 1) * NQT], in_=ov[:])

    nc.compile()
    return nc


_NC_CACHE = None


def _host_inputs(x, offset, weight, bias):
    """Per-core input maps (core b <- batch b) + replicated constants."""
    wq = np.ascontiguousarray(weight, np.float32)  # [COUT, CIN, KH, KW]
    wmat = wq.reshape(COUT, CIN, K).transpose(1, 2, 0).reshape(CIN, K * COUT)
    wmat = np.ascontiguousarray(wmat).astype(ml_dtypes.bfloat16)
    bias_h = np.ascontiguousarray(bias, np.float32).reshape(P, 1)
    ident = np.eye(P, dtype=ml_dtypes.bfloat16)

    ho = (np.arange(NPOS, dtype=np.int32) // WO).astype(np.float32)
    wo = (np.arange(NPOS, dtype=np.int32) % WO).astype(np.float32)
    ky = (np.arange(K, dtype=np.int32) // 3 - 1).astype(np.float32)
    kx = (np.arange(K, dtype=np.int32) % 3 - 1).astype(np.float32)

    in_maps = []
    for b in range(B):
        img = np.ascontiguousarray(x[b], np.float32).transpose(1, 2, 0)
        XPf = np.zeros((HP, WP, P), np.float32)
        XPf[PADR:PADR + H, PADR:PADR + W] = img
        ext = np.vstack([XPf.reshape(NE, P), np.zeros((WP + 1, P), np.float32)])
        xpm = np.concatenate(
            [ext[0:NE], ext[WP:NE + WP], ext[1:NE + 1], ext[WP + 1:NE + WP + 1]],
            axis=1).astype(ml_dtypes.bfloat16)  # [NE, 512]

        offb = np.ascontiguousarray(offset[b], np.float32).reshape(2 * K, NPOS)
        py = (ky[:, None] + ho[None, :]) + offb[0::2]   # [K, NPOS] f32
        px = (kx[:, None] + wo[None, :]) + offb[1::2]
        y0 = np.floor(py)
        x0 = np.floor(px)
        ly = py - y0
        lx = px - x0
        y0c = np.clip(y0, -PADR, 64.0)
        x0c = np.clip(x0, -PADR, 64.0)
        lin = ((y0c + PADR) * WP + (x0c + PADR)).astype(np.int16)  # [K, NPOS]

        # idxc[s + 16g, k*256 + qt*64 + t] = lin[k, qt*1024 + 16t + s]
        lin_r = lin.reshape(K, 4, 64, 16)             # k qt t s
        idx16 = lin_r.transpose(3, 0, 1, 2).reshape(16, K * 4 * 64)
        idxc = np.ascontiguousarray(np.tile(idx16, (8, 1)))  # [128, 2304]

        # wtr[p, ((k*4+qt)*NB + b)*4 + j] = w_j[k, qt*1024 + b*128 + p]
        w4 = np.stack([(1 - ly) * (1 - lx), ly * (1 - lx),
                       (1 - ly) * lx, ly * lx])        # j k pos
        w4r = w4.reshape(4, K, 4, NB, P)               # j k qt b p
        wtr = np.ascontiguousarray(
            w4r.transpose(4, 1, 2, 3, 0).reshape(P, K * 4 * NB * 4)
        ).astype(ml_dtypes.bfloat16)

        in_maps.append({
            "xpm": xpm,
            "idxc": idxc,
            "wtr": wtr,
            "wmat": wmat,
            "bias": bias_h,
            "ident": ident,
        })
    return in_maps


def kernel(x, offset, weight, bias):
    global _NC_CACHE
    from concourse.bass_utils import run_bass_kernel_spmd

    if _NC_CACHE is None:
        _NC_CACHE = _build_kernel()
    nc = _NC_CACHE
    in_maps = _host_inputs(x, offset, weight, bias)
    res = run_bass_kernel_spmd(nc, in_maps, list(range(B)))
    out = np.stack([res.results[b]["out"].reshape(COUT, HO, WO) for b in range(B)])
    return out.astype(np.float32)


if __name__ == "__main__":
    import sys
    d = np.load("/tmp/inputs.npz")
    if len(sys.argv) > 1 and sys.argv[1] == "sim":
        from concourse.bass_interp import CoreSim
        nc = _build_kernel()
        in_maps = _host_inputs(d["x"], d["offset"], d["weight"], d["bias"])
        sim = CoreSim(nc)
        for kk, vv in in_maps[0].items():
            sim.tensor(kk)[:] = vv
        sim.simulate()
        out = np.asarray(sim.tensor("out")).reshape(1, COUT, HO, WO)
        exp = np.load("/tmp/expected.npy")[0:1]
    else:
        out = kernel(d["x"], d["offset"], d["weight"], d["bias"])
        exp = np.load("/tmp/expected.npy")
    err = np.abs(out - exp)
    print("rel l2:", np.linalg.norm(out - exp) / np.linalg.norm(exp))
    print("absmax rel:", err.max() / np.abs(exp).max())
